# revision 7
# baseline (speedup 1.0000x reference)
"""Expert-parallel MoE SwiGLU kernel for 8 Trainium2 NeuronCores.

Strategy: expert parallelism with host-side dispatch/combine. Each of the
8 cores owns one expert's weights. The host routes tokens by expert_idx,
packs each expert's tokens as a transposed [D, W] panel (features on
partitions so no on-chip transposes are needed anywhere), and each core
runs a dense SwiGLU FFN:  yT = w_down.T-blocks @ (silu(wg.T@xT) * (wu.T@xT)).
Matmul operands stream as fp16 (fp32 PSUM accumulation; ~6e-4 max
relative error vs the fp32 reference), halving the weight traffic.

v2: weights are host-repacked into large contiguous panels so the whole
kernel needs only ~40 DMA instructions (vs ~112 tile-sized ones). The
per-DMA ~660ns issue cost on the sync engine serialized the baseline's
startup: weights could not arrive fast enough for the first ~20us and
the PE ran throttled/starved. Chunked 0.5MB group DMAs + x first on the
sync queue let real matmuls start at ~9us and run back-to-back at full
clock. y writeback is issued per-output-tile during the final down-
projection so the tail costs only the last tile's DMA + barrier.
"""

import numpy as np
from contextlib import ExitStack

D_MODEL = 1024
D_FF = 4096
N_EXPERTS = 8
N_CORES = 8

_ND = D_MODEL // 128   # 8 contraction chunks over d_model
_NF = D_FF // 128      # 32 f chunks
_NG = 4                # weight streaming groups over d_ff
_FPG = _NF // _NG      # 8 f-tiles per group
_NCH = 4               # DMA chunks per (matrix, group): each = 2 f-tiles
_CHW = _FPG // _NCH * D_MODEL  # 2048 cols per chunk (f-major packed)

_nc_cache = {}

# warmup dummies: keep the PE activity monitor busy while the first
# weight chunks land so real matmuls run at 2.4GHz, not the cold clock.
# any coverage gap >~0.5us resets HAM's free-running 3.4us busy window,
# so the dummy run must seamlessly hand off to the first real matmul.
_WARM0 = 8
_WARMI = 2

import os as _os
_CDT = _os.environ.get("MOE_KERNEL_DTYPE", "float16")


def _np_cdt():
    if _CDT == "bfloat16":
        import ml_dtypes
        return ml_dtypes.bfloat16
    return np.float16


def _build_nc(W: int):
    """Build + schedule the per-core Bass program for token capacity W."""
    import concourse.bacc as bacc
    import concourse.tile as tile
    from concourse import mybir

    f32 = mybir.dt.float32
    f16 = getattr(mybir.dt, _CDT if _CDT != "float16" else "float16")

    NW = _ND * W

    nc = bacc.Bacc("TRN2", target_bir_lowering=False, debug=False,
                   num_devices=N_CORES)
    xt = nc.dram_tensor("xt", [128, NW], f16, kind="ExternalInput").ap()
    wg = nc.dram_tensor("wg", [_NG, _NCH, 128, _CHW], f16,
                        kind="ExternalInput").ap()
    wu = nc.dram_tensor("wu", [_NG, _NCH, 128, _CHW], f16,
                        kind="ExternalInput").ap()
    wd = nc.dram_tensor("wd", [_NG, 128, _FPG * D_MODEL], f16,
                        kind="ExternalInput").ap()
    yt = nc.dram_tensor("yt", [128, NW], f32, kind="ExternalOutput").ap()

    with tile.TileContext(nc) as tc, ExitStack() as ctx:
        xpool = ctx.enter_context(tc.tile_pool(name="x", bufs=1))
        wgp = ctx.enter_context(tc.tile_pool(name="wgp", bufs=2))
        wup = ctx.enter_context(tc.tile_pool(name="wup", bufs=2))
        w0p = ctx.enter_context(tc.tile_pool(name="w0p", bufs=1))
        wdp = ctx.enter_context(tc.tile_pool(name="wdp", bufs=2))
        tp = ctx.enter_context(tc.tile_pool(name="tp", bufs=2))
        gap = ctx.enter_context(tc.tile_pool(name="gap", bufs=3))
        yp = ctx.enter_context(tc.tile_pool(name="yp", bufs=1))
        pg = ctx.enter_context(tc.tile_pool(name="pg", bufs=2, space="PSUM"))
        pu = ctx.enter_context(tc.tile_pool(name="pu", bufs=2, space="PSUM"))
        pd = ctx.enter_context(tc.tile_pool(name="pd", bufs=4, space="PSUM"))

        # x panel first on the sync DMA queue: needed by every matmul.
        # Two tiles (d 0-3 / d 4-7) so the first gate matmuls only wait
        # for the first half.
        HX = _ND // 2
        x_a = xpool.tile([128, HX * W], f16, tag="xa")
        x_b = xpool.tile([128, HX * W], f16, tag="xb")
        nc.sync.dma_start(x_a[:], xt[:, :HX * W])

        def x_mov(d):
            t = x_a if d < HX else x_b
            return t[:, (d % HX) * W:(d % HX + 1) * W]

        y_acc = [yp.tile([128, W], f32, tag=f"y{d}", name=f"y_acc{d}")
                 for d in range(_ND)]

        # HAM warm-up scratch (full-width dummies; see baseline notes).
        scr_w = xpool.tile([128, 128], f16, tag="scrw", name="scr_w")
        scr_x = xpool.tile([128, W], f16, tag="scrx", name="scr_x")
        nc.vector.memset(scr_w[:], 0.0)
        nc.vector.memset(scr_x[:], 0.0)
        scr_p = pd.tile([128, W], f32, tag="pd", name="scr_p")
        scr_p2 = pd.tile([128, W], f32, tag="pd", name="scr_p2")
        _scr = [scr_p, scr_p2]

        def emit_warmup(n):
            for i in range(n):
                nc.tensor.matmul(_scr[i % 2][:], scr_w[:], scr_x[:],
                                 start=True, stop=True)

        def emit_down(g, t_tiles, wd_t, dts):
            # y[dt] += wd[g rows, dt cols].T @ t   for dt in dts
            for dt in dts:
                pdt = pd.tile([128, W], f32, tag="pd", name=f"pd_{g}_{dt}")
                for ft in range(_FPG):
                    nc.tensor.matmul(
                        pdt[:],
                        wd_t[:, ft * D_MODEL + dt * 128:
                             ft * D_MODEL + (dt + 1) * 128],
                        t_tiles[ft][:],
                        start=(ft == 0), stop=(ft == _FPG - 1))
                if g == 0:
                    nc.vector.tensor_copy(y_acc[dt][:], pdt[:])
                else:
                    nc.vector.tensor_add(y_acc[dt][:], y_acc[dt][:], pdt[:])

        prev = None  # (g, t_tiles, wd_t) of the previous group
        for g in range(_NG):
            # group DMAs: chunks alternate gate/up so the earliest f-tiles
            # unblock first; wd[g] (consumed during group g+1) last.
            # Group 0 streams in single-f-tile pieces on dedicated tiles so
            # the very first real matmul only waits for x_a + one 0.25MB
            # piece; later groups use 2-f-tile chunks.
            wg_t, wu_t = [], []
            if g == 0:
                for ft in range(_FPG):
                    g_t = w0p.tile([128, D_MODEL], f16, tag=f"g0_{ft}")
                    nc.sync.dma_start(
                        g_t[:], wg[0, ft // 2, :,
                                   (ft % 2) * D_MODEL:(ft % 2 + 1) * D_MODEL])
                    wg_t.append(g_t)
                    if ft == 0:
                        nc.sync.dma_start(x_b[:], xt[:, HX * W:])
                    u_t = w0p.tile([128, D_MODEL], f16, tag=f"u0_{ft}")
                    nc.sync.dma_start(
                        u_t[:], wu[0, ft // 2, :,
                                   (ft % 2) * D_MODEL:(ft % 2 + 1) * D_MODEL])
                    wu_t.append(u_t)
            else:
                for c in range(_NCH):
                    g_t = wgp.tile([128, _CHW], f16, tag=f"g{c}")
                    nc.sync.dma_start(g_t[:], wg[g, c])
                    wg_t.append(g_t)
                    u_t = wup.tile([128, _CHW], f16, tag=f"u{c}")
                    nc.sync.dma_start(u_t[:], wu[g, c])
                    wu_t.append(u_t)
            wd_t = wdp.tile([128, _FPG * D_MODEL], f16, tag="wd")
            nc.sync.dma_start(wd_t[:], wd[g])

            if g == 0:
                emit_warmup(_WARM0)

            if g == 0:
                def g_stat(ft, d):
                    return wg_t[ft][:, d * 128:(d + 1) * 128]

                def u_stat(ft, d):
                    return wu_t[ft][:, d * 128:(d + 1) * 128]
            else:
                def g_stat(ft, d):
                    return wg_t[ft // 2][:, (ft % 2) * D_MODEL + d * 128:
                                         (ft % 2) * D_MODEL + (d + 1) * 128]

                def u_stat(ft, d):
                    return wu_t[ft // 2][:, (ft % 2) * D_MODEL + d * 128:
                                         (ft % 2) * D_MODEL + (d + 1) * 128]

            t_tiles = []
            for ft in range(_FPG):
                if g == 0 and ft < 2:
                    emit_warmup(_WARMI)
                psg = pg.tile([128, W], f32)
                for d in range(_ND):
                    nc.tensor.matmul(psg[:], g_stat(ft, d), x_mov(d),
                                     start=(d == 0), stop=(d == _ND - 1))
                psu = pu.tile([128, W], f32)
                for d in range(_ND):
                    nc.tensor.matmul(psu[:], u_stat(ft, d), x_mov(d),
                                     start=(d == 0), stop=(d == _ND - 1))
                g_act = gap.tile([128, W], f32, tag="gact")
                nc.scalar.activation(g_act[:], psg[:],
                                     mybir.ActivationFunctionType.Silu)
                t_t = tp.tile([128, W], f16, tag=f"t{ft}")
                nc.vector.tensor_mul(t_t[:], g_act[:], psu[:])
                t_tiles.append(t_t)
                if prev is not None:
                    emit_down(prev[0], prev[1], prev[2], (ft,))
            prev = (g, t_tiles, wd_t)

        # final group's down-projection; kick each output tile's DMA as
        # soon as its accumulation completes so writeback overlaps compute
        for dt in range(_ND):
            emit_down(prev[0], prev[1], prev[2], (dt,))
            nc.scalar.dma_start(yt[:, dt * W:(dt + 1) * W], y_acc[dt][:])

    nc.compile()
    return nc


def _pack_gu(w):
    # [D, F] -> [NG, NCH, 128, CHW]; f-major within a group so the first
    # DMA chunk is exactly the first f-tiles (all d-chunks)
    a = np.asarray(w).astype(_np_cdt())
    a = a.reshape(_ND, 128, _NG, _FPG, 128)      # [d, p, g, ft, f']
    a = a.transpose(2, 1, 3, 0, 4)               # [g, p, ft, d, f']
    a = a.reshape(_NG, 128, _FPG * D_MODEL)      # col = ft*1024 + d*128 + f'
    a = a.reshape(_NG, 128, _NCH, _CHW).transpose(0, 2, 1, 3)
    return np.ascontiguousarray(a)               # [g, c, p, CHW]


def _pack_wd(w):
    # [F, D] -> [NG, 128, FPG*D]; col = ft*1024 + j
    a = np.asarray(w).astype(_np_cdt())
    a = a.reshape(_NG, _FPG, 128, D_MODEL).transpose(0, 2, 1, 3)
    return np.ascontiguousarray(a.reshape(_NG, 128, _FPG * D_MODEL))


def _run_one(W, tok_lists, x_flat, packed_w, out_flat):
    from concourse.bass_utils import run_bass_kernel_spmd

    if W not in _nc_cache:
        _nc_cache[W] = _build_nc(W)
    nc = _nc_cache[W]

    cdt = _np_cdt()
    in_maps = []
    for e in range(N_EXPERTS):
        toks = tok_lists[e]
        xp = np.zeros((_ND, 128, W), dtype=cdt)
        if len(toks):
            xp[:, :, :len(toks)] = (
                x_flat[toks].T.astype(cdt).reshape(_ND, 128, len(toks)))
        in_maps.append({
            "xt": np.ascontiguousarray(
                xp.transpose(1, 0, 2).reshape(128, _ND * W)),
            "wg": packed_w[e][0],
            "wu": packed_w[e][1],
            "wd": packed_w[e][2],
        })

    res = None
    for attempt in range(3):
        try:
            res = run_bass_kernel_spmd(nc, in_maps,
                                       core_ids=list(range(N_CORES)))
            break
        except Exception:
            if attempt == 2:
                raise
            import time
            time.sleep(3.0)
            try:
                import jax
                jax.clear_caches()
                jax.clear_backends()
            except Exception:
                pass
    for e in range(N_EXPERTS):
        toks = tok_lists[e]
        if not len(toks):
            continue
        y = res.results[e]["yt"].reshape(128, _ND, W).transpose(1, 0, 2)
        out_flat[toks] = y.reshape(D_MODEL, W)[:, :len(toks)].T


def kernel(x, expert_idx, w_gate, w_up, w_down):
    x = np.asarray(x, dtype=np.float32)
    idx = np.asarray(expert_idx).astype(np.int64)
    B, S, D = x.shape
    T = B * S
    x_flat = np.ascontiguousarray(x.reshape(T, D))
    idx_flat = idx.reshape(T)

    packed_w = [
        (_pack_gu(w_gate[e]), _pack_gu(w_up[e]), _pack_wd(w_down[e]))
        for e in range(N_EXPERTS)
    ]

    tok_lists = [np.nonzero(idx_flat == e)[0] for e in range(N_EXPERTS)]
    cap = max(1, max(len(t) for t in tok_lists))
    out_flat = np.zeros((T, D), dtype=np.float32)

    if cap <= 512:
        # normal path: one SPMD run, capacity = max expert load
        W = max(256, cap)
        _run_one(W, tok_lists, x_flat, packed_w, out_flat)
    else:
        # fallback for extreme routing imbalance: rounds of <=512/expert
        rounds = -(-cap // 512)
        for r in range(rounds):
            round_lists = [t[r * 512:(r + 1) * 512] for t in tok_lists]
            _run_one(512, round_lists, x_flat, packed_w, out_flat)

    return out_flat.reshape(B, S, D)


# revision 11
# speedup vs baseline: 1.2355x; 1.2355x over previous
"""Expert-parallel MoE SwiGLU kernel for 8 Trainium2 NeuronCores.

Strategy: expert parallelism with host-side dispatch/combine, plus 2-way
intra-expert parallelism for load balance. Experts are sorted by token
count and paired heavy-with-light; each pair occupies two cores. A core
runs TWO half-FFNs ("slot A" = the heavy expert, "slot B" = the light
one), each over half of d_ff (2048): core 2p gets (heavy, F-half 0) and
(light, F-half 1), core 2p+1 the complements. Each expert's full output
is the fp32 sum of its two half-F partials, combined on the host. This
pads the token panels to max(heavy) + max(light) columns instead of
2*max(all), cutting PE work ~7% under typical routing imbalance.

Per half-FFN the math is the baseline's: tokens packed as a transposed
[D, W] panel (features on partitions, no on-chip transposes);
yT = w_down.T-blocks @ (silu(wg.T@xT) * (wu.T@xT)); operands in fp16
with fp32 PSUM accumulation (~6e-4 max rel err).

DMA design (v2): weights are host-repacked into contiguous f-major
panels so the whole kernel needs ~40 DMA instructions; the per-DMA
~660ns issue cost on the sync engine otherwise starves the PE for the
first ~20us. 0.5MB chunk DMAs arrive in consumption order; x panels
ride the same queue ahead of the weights they feed. Dummy warmup
matmuls bridge the DMA cold-start so the HAM activity window never
breaks and the clock stays at 2.4GHz. y writeback is issued per output
tile as its accumulation finishes (slot A's during slot B's compute),
leaving only the last tile's DMA + exit barrier on the tail.
"""

import numpy as np
from contextlib import ExitStack

D_MODEL = 1024
D_FF = 4096
N_EXPERTS = 8
N_CORES = 8

_ND = D_MODEL // 128    # 8 contraction chunks over d_model
_FH = D_FF // 2         # 2048: f columns per slot (half expert)
_NG = _FH // 1024       # 2 weight streaming groups per slot
_FPG = 8                # f-tiles per group
_NCH = 4                # DMA chunks per (matrix, group): each = 2 f-tiles
_CHW = 2048             # cols per chunk

_nc_cache = {}

# warmup dummies: keep the PE activity monitor busy while the first
# weight chunks land so real matmuls run at 2.4GHz, not the cold clock.
# any coverage gap >~0.5us can reset HAM's free-running 3.4us busy
# window, so the dummies must seamlessly hand off to the first real MM.
_WARM0 = 17
_WARMI = 2

import os as _os
_CDT = _os.environ.get("MOE_KERNEL_DTYPE", "float16")


def _np_cdt():
    if _CDT == "bfloat16":
        import ml_dtypes
        return ml_dtypes.bfloat16
    return np.float16


def _build_nc(WA: int, WB: int):
    """Per-core Bass program: slot A (width WA) then slot B (width WB),
    each a half-F SwiGLU FFN, fully pipelined."""
    import concourse.bacc as bacc
    import concourse.tile as tile
    from concourse import mybir

    f32 = mybir.dt.float32
    f16 = getattr(mybir.dt, _CDT if _CDT != "float16" else "float16")

    nc = bacc.Bacc("TRN2", target_bir_lowering=False, debug=False,
                   num_devices=N_CORES)

    dram = {}
    for s, Ws in (("a", WA), ("b", WB)):
        dram[f"xt{s}"] = nc.dram_tensor(f"xt{s}", [128, _ND * Ws], f16,
                                        kind="ExternalInput").ap()
        dram[f"wg{s}"] = nc.dram_tensor(f"wg{s}", [_NG, _NCH, 128, _CHW],
                                        f16, kind="ExternalInput").ap()
        dram[f"wu{s}"] = nc.dram_tensor(f"wu{s}", [_NG, _NCH, 128, _CHW],
                                        f16, kind="ExternalInput").ap()
        dram[f"wd{s}"] = nc.dram_tensor(f"wd{s}", [_NG, 128, _FPG * D_MODEL],
                                        f16, kind="ExternalInput").ap()
        dram[f"yt{s}"] = nc.dram_tensor(f"yt{s}", [128, _ND * Ws], f32,
                                        kind="ExternalOutput").ap()

    with tile.TileContext(nc) as tc, ExitStack() as ctx:
        xpool = ctx.enter_context(tc.tile_pool(name="x", bufs=1))
        wgp = ctx.enter_context(tc.tile_pool(name="wgp", bufs=2))
        wup = ctx.enter_context(tc.tile_pool(name="wup", bufs=2))
        wdp = ctx.enter_context(tc.tile_pool(name="wdp", bufs=2))
        tp = ctx.enter_context(tc.tile_pool(name="tp", bufs=2))
        gap = ctx.enter_context(tc.tile_pool(name="gap", bufs=3))
        yp = ctx.enter_context(tc.tile_pool(name="yp", bufs=1))
        pg = ctx.enter_context(tc.tile_pool(name="pg", bufs=2, space="PSUM"))
        pu = ctx.enter_context(tc.tile_pool(name="pu", bufs=2, space="PSUM"))
        pd = ctx.enter_context(tc.tile_pool(name="pd", bufs=4, space="PSUM"))

        # slot A's x panel first on the sync queue: it gates everything.
        x_t = {"a": xpool.tile([128, _ND * WA], f16, tag="xa",
                       name="x_a")}
        nc.sync.dma_start(x_t["a"][:], dram["xta"][:])

        y_acc = {s: [yp.tile([128, Ws], f32, tag=f"y{s}{d}",
                             name=f"y_{s}{d}")
                     for d in range(_ND)]
                 for s, Ws in (("a", WA), ("b", WB))}

        # HAM warm-up scratch (full-width dummies).
        scr_w = xpool.tile([128, 128], f16, tag="scrw", name="scr_w")
        scr_x = xpool.tile([128, WA], f16, tag="scrx", name="scr_x")
        nc.vector.memset(scr_w[:], 0.0)
        nc.vector.memset(scr_x[:], 0.0)
        scr_p = pd.tile([128, WA], f32, tag="pd", name="scr_p")
        scr_p2 = pd.tile([128, WA], f32, tag="pd", name="scr_p2")
        _scr = [scr_p, scr_p2]

        def emit_warmup(n):
            for i in range(n):
                nc.tensor.matmul(_scr[i % 2][:], scr_w[:], scr_x[:],
                                 start=True, stop=True)

        def emit_down(prev, dts):
            # y[dt] += wd[group rows, dt cols].T @ t ; on the slot's final
            # group also kick the output DMA for that tile immediately.
            s, Ws, g, t_tiles, wd_t = prev
            for dt in dts:
                pdt = pd.tile([128, WA], f32, tag="pd", name=f"pd{s}{g}{dt}")
                for ft in range(_FPG):
                    nc.tensor.matmul(
                        pdt[:, :Ws],
                        wd_t[:, ft * D_MODEL + dt * 128:
                             ft * D_MODEL + (dt + 1) * 128],
                        t_tiles[ft][:],
                        start=(ft == 0), stop=(ft == _FPG - 1))
                if g == 0:
                    nc.vector.tensor_copy(y_acc[s][dt][:], pdt[:, :Ws])
                else:
                    nc.vector.tensor_add(y_acc[s][dt][:], y_acc[s][dt][:],
                                         pdt[:, :Ws])
                    nc.scalar.dma_start(
                        dram[f"yt{s}"][:, dt * Ws:(dt + 1) * Ws],
                        y_acc[s][dt][:])

        prev = None
        for s, Ws in (("a", WA), ("b", WB)):
            for g in range(_NG):
                # group DMAs: chunks alternate gate/up in consumption
                # order; wd[g] (consumed one group later) last. Slot B's
                # x panel rides after slot A group 0's weights.
                wg_t, wu_t = [], []
                for c in range(_NCH):
                    g_t = wgp.tile([128, _CHW], f16, tag=f"g{c}")
                    nc.sync.dma_start(g_t[:], dram[f"wg{s}"][g, c])
                    wg_t.append(g_t)
                    u_t = wup.tile([128, _CHW], f16, tag=f"u{c}")
                    nc.sync.dma_start(u_t[:], dram[f"wu{s}"][g, c])
                    wu_t.append(u_t)
                wd_t = wdp.tile([128, _FPG * D_MODEL], f16, tag="wd")
                nc.sync.dma_start(wd_t[:], dram[f"wd{s}"][g])
                if s == "a" and g == 0:
                    x_t["b"] = xpool.tile([128, _ND * WB], f16, tag="xb",
                                          name="x_b")
                    nc.sync.dma_start(x_t["b"][:], dram["xtb"][:])
                    emit_warmup(_WARM0)

                def g_stat(ft, d):
                    return wg_t[ft // 2][:, (ft % 2) * D_MODEL + d * 128:
                                         (ft % 2) * D_MODEL + (d + 1) * 128]

                def u_stat(ft, d):
                    return wu_t[ft // 2][:, (ft % 2) * D_MODEL + d * 128:
                                         (ft % 2) * D_MODEL + (d + 1) * 128]

                def x_mov(d):
                    return x_t[s][:, d * Ws:(d + 1) * Ws]

                t_tiles = []
                for ft in range(_FPG):
                    if s == "a" and g == 0 and ft < 2:
                        emit_warmup(_WARMI)
                    psg = pg.tile([128, WA], f32)
                    for d in range(_ND):
                        nc.tensor.matmul(psg[:, :Ws], g_stat(ft, d),
                                         x_mov(d),
                                         start=(d == 0), stop=(d == _ND - 1))
                    psu = pu.tile([128, WA], f32)
                    for d in range(_ND):
                        nc.tensor.matmul(psu[:, :Ws], u_stat(ft, d),
                                         x_mov(d),
                                         start=(d == 0), stop=(d == _ND - 1))
                    g_act = gap.tile([128, WA], f32, tag="gact")
                    nc.scalar.activation(g_act[:, :Ws], psg[:, :Ws],
                                         mybir.ActivationFunctionType.Silu)
                    t_t = tp.tile([128, Ws], f16, tag=f"t{s}{ft}")
                    nc.vector.tensor_mul(t_t[:], g_act[:, :Ws], psu[:, :Ws])
                    t_tiles.append(t_t)
                    if prev is not None:
                        emit_down(prev, (ft,))
                prev = (s, Ws, g, t_tiles, wd_t)

        # last group's down-projection (slot B group 1)
        for dt in range(_ND):
            emit_down(prev, (dt,))

    nc.compile()
    return nc


def _pack_gu(w):
    # [D, F] -> [F//1024, NCH, 128, CHW]; f-major within each group so
    # chunks arrive in exactly the order the f-tile loop consumes them
    a = np.asarray(w).astype(_np_cdt())
    D, F = a.shape
    ng = F // (_FPG * 128)
    a = a.reshape(_ND, 128, ng, _FPG, 128)       # [d, p, g, ft, f']
    a = a.transpose(2, 1, 3, 0, 4)               # [g, p, ft, d, f']
    a = a.reshape(ng, 128, _FPG * D_MODEL)       # col = ft*1024 + d*128 + f'
    a = a.reshape(ng, 128, _NCH, _CHW).transpose(0, 2, 1, 3)
    return np.ascontiguousarray(a)               # [g, c, p, CHW]


def _pack_wd(w):
    # [F, D] -> [F//1024, 128, FPG*D]; col = ft*1024 + j
    a = np.asarray(w).astype(_np_cdt())
    F, D = a.shape
    ng = F // (_FPG * 128)
    a = a.reshape(ng, _FPG, 128, D_MODEL).transpose(0, 2, 1, 3)
    return np.ascontiguousarray(a.reshape(ng, 128, _FPG * D_MODEL))


def _pack_x_panel(x_flat, toks, Ws):
    cdt = _np_cdt()
    xp = np.zeros((_ND, 128, Ws), dtype=cdt)
    if len(toks):
        xp[:, :, :len(toks)] = (
            x_flat[toks].T.astype(cdt).reshape(_ND, 128, len(toks)))
    return np.ascontiguousarray(xp.transpose(1, 0, 2).reshape(128, _ND * Ws))


def _plan(tok_lists):
    """Pair heavy experts with light ones; 2 cores per pair."""
    counts = np.array([len(t) for t in tok_lists])
    order = np.argsort(-counts, kind="stable")
    pairs = [(int(order[i]), int(order[N_EXPERTS - 1 - i]))
             for i in range(N_EXPERTS // 2)]
    WA = max(192, int(counts[order[0]]))
    WB = max(192, int(counts[order[N_EXPERTS // 2]]))
    return pairs, WA, WB


def _make_in_maps(pairs, WA, WB, tok_lists, x_flat, packed_w):
    """Core 2p: (heavy_p, F-half 0) + (light_p, F-half 1);
    core 2p+1 the complementary halves. packed_w[e] = (wg2, wu2, wd2)
    with the group axis spanning both halves (4 groups of 1024 f-cols:
    halves = groups [0:2] and [2:4])."""
    in_maps = []
    for p, (ea, eb) in enumerate(pairs):
        xa = _pack_x_panel(x_flat, tok_lists[ea], WA)
        xb = _pack_x_panel(x_flat, tok_lists[eb], WB)
        for half in (0, 1):
            ga, ua, da = packed_w[ea]
            gb, ub, db = packed_w[eb]
            ha = slice(2 * half, 2 * half + 2)
            hb = slice(2 * (1 - half), 2 * (1 - half) + 2)
            in_maps.append({
                "xta": xa, "xtb": xb,
                "wga": ga[ha], "wua": ua[ha], "wda": da[ha],
                "wgb": gb[hb], "wub": ub[hb], "wdb": db[hb],
            })
    return in_maps


def _combine(res, pairs, WA, WB, tok_lists, out_flat):
    for p, (ea, eb) in enumerate(pairs):
        ya = (np.asarray(res[2 * p]["yta"], dtype=np.float32)
              + np.asarray(res[2 * p + 1]["yta"], dtype=np.float32))
        yb = (np.asarray(res[2 * p]["ytb"], dtype=np.float32)
              + np.asarray(res[2 * p + 1]["ytb"], dtype=np.float32))
        for e, y, Ws in ((ea, ya, WA), (eb, yb, WB)):
            toks = tok_lists[e]
            if not len(toks):
                continue
            yf = y.reshape(128, _ND, Ws).transpose(1, 0, 2)
            out_flat[toks] = yf.reshape(D_MODEL, Ws)[:, :len(toks)].T


def _run_one(pairs, WA, WB, tok_lists, x_flat, packed_w, out_flat):
    from concourse.bass_utils import run_bass_kernel_spmd

    key = (WA, WB)
    if key not in _nc_cache:
        _nc_cache[key] = _build_nc(WA, WB)
    nc = _nc_cache[key]

    in_maps = _make_in_maps(pairs, WA, WB, tok_lists, x_flat, packed_w)

    res = None
    for attempt in range(3):
        try:
            res = run_bass_kernel_spmd(nc, in_maps,
                                       core_ids=list(range(N_CORES)))
            break
        except Exception:
            if attempt == 2:
                raise
            import time
            time.sleep(3.0)
            try:
                import jax
                jax.clear_caches()
                jax.clear_backends()
            except Exception:
                pass
    _combine(res.results, pairs, WA, WB, tok_lists, out_flat)


def kernel(x, expert_idx, w_gate, w_up, w_down):
    x = np.asarray(x, dtype=np.float32)
    idx = np.asarray(expert_idx).astype(np.int64)
    B, S, D = x.shape
    T = B * S
    x_flat = np.ascontiguousarray(x.reshape(T, D))
    idx_flat = idx.reshape(T)

    packed_w = [
        (_pack_gu(w_gate[e]), _pack_gu(w_up[e]), _pack_wd(w_down[e]))
        for e in range(N_EXPERTS)
    ]

    tok_lists = [np.nonzero(idx_flat == e)[0] for e in range(N_EXPERTS)]
    cap = max(1, max(len(t) for t in tok_lists))
    out_flat = np.zeros((T, D), dtype=np.float32)

    if cap <= 512:
        pairs, WA, WB = _plan(tok_lists)
        _run_one(pairs, WA, WB, tok_lists, x_flat, packed_w, out_flat)
    else:
        # fallback for extreme routing imbalance: rounds of <=512/expert
        rounds = -(-cap // 512)
        pairs = [(i, N_EXPERTS - 1 - i) for i in range(N_EXPERTS // 2)]
        for r in range(rounds):
            round_lists = [t[r * 512:(r + 1) * 512] for t in tok_lists]
            _run_one(pairs, 512, 512, round_lists, x_flat, packed_w,
                     out_flat)

    return out_flat.reshape(B, S, D)


# revision 12
# speedup vs baseline: 1.2535x; 1.0146x over previous
"""Expert-parallel MoE SwiGLU kernel for 8 Trainium2 NeuronCores.

Strategy: expert parallelism with host-side dispatch/combine, plus 2-way
intra-expert parallelism for load balance. Experts are sorted by token
count and paired heavy-with-light; each pair occupies two cores. A core
runs TWO half-FFNs ("slot A" = the heavy expert, "slot B" = the light
one), each over half of d_ff (2048): core 2p gets (heavy, F-half 0) and
(light, F-half 1), core 2p+1 the complements. Each expert's full output
is the fp32 sum of its two half-F partials, combined on the host. This
pads the token panels to max(heavy) + max(light) columns instead of
2*max(all), cutting PE work ~7% under typical routing imbalance.

Per half-FFN the math is the baseline's: tokens packed as a transposed
[D, W] panel (features on partitions, no on-chip transposes);
yT = w_down.T-blocks @ (silu(wg.T@xT) * (wu.T@xT)); operands in fp16
with fp32 PSUM accumulation (~6e-4 max rel err).

DMA design (v2): weights are host-repacked into contiguous f-major
panels so the whole kernel needs ~40 DMA instructions; the per-DMA
~660ns issue cost on the sync engine otherwise starves the PE for the
first ~20us. 0.5MB chunk DMAs arrive in consumption order; x panels
ride the same queue ahead of the weights they feed. Dummy warmup
matmuls bridge the DMA cold-start so the HAM activity window never
breaks and the clock stays at 2.4GHz. y writeback is issued per output
tile as its accumulation finishes (slot A's during slot B's compute),
leaving only the last tile's DMA + exit barrier on the tail.
"""

import numpy as np
from contextlib import ExitStack

D_MODEL = 1024
D_FF = 4096
N_EXPERTS = 8
N_CORES = 8

_ND = D_MODEL // 128    # 8 contraction chunks over d_model
_FH = D_FF // 2         # 2048: f columns per slot (half expert)
_NG = _FH // 1024       # 2 weight streaming groups per slot
_FPG = 8                # f-tiles per group
_NCH = 4                # DMA chunks per (matrix, group): each = 2 f-tiles
_CHW = 2048             # cols per chunk

_nc_cache = {}

# warmup dummies: keep the PE activity monitor busy while the first
# weight chunks land so real matmuls run at 2.4GHz, not the cold clock.
# any coverage gap >~0.5us can reset HAM's free-running 3.4us busy
# window, so the dummies must seamlessly hand off to the first real MM.
_WARM0 = 21
_WARMI = 2

import os as _os
_CDT = _os.environ.get("MOE_KERNEL_DTYPE", "float16")


def _np_cdt():
    if _CDT == "bfloat16":
        import ml_dtypes
        return ml_dtypes.bfloat16
    return np.float16


def _build_nc(WA: int, WB: int):
    """Per-core Bass program: slot A (width WA) then slot B (width WB),
    each a half-F SwiGLU FFN, fully pipelined."""
    import concourse.bacc as bacc
    import concourse.tile as tile
    from concourse import mybir

    f32 = mybir.dt.float32
    f16 = getattr(mybir.dt, _CDT if _CDT != "float16" else "float16")

    nc = bacc.Bacc("TRN2", target_bir_lowering=False, debug=False,
                   num_devices=N_CORES)

    dram = {}
    for s, Ws in (("a", WA), ("b", WB)):
        dram[f"xt{s}"] = nc.dram_tensor(f"xt{s}", [128, _ND * Ws], f16,
                                        kind="ExternalInput").ap()
        dram[f"wg{s}"] = nc.dram_tensor(f"wg{s}", [_NG, _NCH, 128, _CHW],
                                        f16, kind="ExternalInput").ap()
        dram[f"wu{s}"] = nc.dram_tensor(f"wu{s}", [_NG, _NCH, 128, _CHW],
                                        f16, kind="ExternalInput").ap()
        dram[f"wd{s}"] = nc.dram_tensor(f"wd{s}", [_NG, 128, _FPG * D_MODEL],
                                        f16, kind="ExternalInput").ap()
        dram[f"yt{s}"] = nc.dram_tensor(f"yt{s}", [128, _ND * Ws], f32,
                                        kind="ExternalOutput").ap()

    with tile.TileContext(nc) as tc, ExitStack() as ctx:
        xpool = ctx.enter_context(tc.tile_pool(name="x", bufs=1))
        wgp = ctx.enter_context(tc.tile_pool(name="wgp", bufs=2))
        wup = ctx.enter_context(tc.tile_pool(name="wup", bufs=2))
        wdp = ctx.enter_context(tc.tile_pool(name="wdp", bufs=2))
        w0p = ctx.enter_context(tc.tile_pool(name="w0p", bufs=1))
        tp = ctx.enter_context(tc.tile_pool(name="tp", bufs=2))
        gap = ctx.enter_context(tc.tile_pool(name="gap", bufs=3))
        yp = ctx.enter_context(tc.tile_pool(name="yp", bufs=1))
        pg = ctx.enter_context(tc.tile_pool(name="pg", bufs=2, space="PSUM"))
        pu = ctx.enter_context(tc.tile_pool(name="pu", bufs=2, space="PSUM"))
        pd = ctx.enter_context(tc.tile_pool(name="pd", bufs=4, space="PSUM"))

        # slot A's x panel first on the sync queue: it gates everything.
        x_t = {"a": xpool.tile([128, _ND * WA], f16, tag="xa",
                       name="x_a")}
        nc.sync.dma_start(x_t["a"][:], dram["xta"][:])

        y_acc = {s: [yp.tile([128, Ws], f32, tag=f"y{s}{d}",
                             name=f"y_{s}{d}")
                     for d in range(_ND)]
                 for s, Ws in (("a", WA), ("b", WB))}

        # HAM warm-up scratch (full-width dummies).
        scr_w = xpool.tile([128, 128], f16, tag="scrw", name="scr_w")
        scr_x = xpool.tile([128, WA], f16, tag="scrx", name="scr_x")
        nc.vector.memset(scr_w[:], 0.0)
        nc.vector.memset(scr_x[:], 0.0)
        scr_p = pd.tile([128, WA], f32, tag="pd", name="scr_p")
        scr_p2 = pd.tile([128, WA], f32, tag="pd", name="scr_p2")
        _scr = [scr_p, scr_p2]

        def emit_warmup(n):
            for i in range(n):
                nc.tensor.matmul(_scr[i % 2][:], scr_w[:], scr_x[:],
                                 start=True, stop=True)

        def emit_down(prev, dts):
            # y[dt] += wd[group rows, dt cols].T @ t ; on the slot's final
            # group also kick the output DMA for that tile immediately.
            s, Ws, g, t_tiles, wd_t = prev
            for dt in dts:
                pdt = pd.tile([128, WA], f32, tag="pd", name=f"pd{s}{g}{dt}")
                for ft in range(_FPG):
                    nc.tensor.matmul(
                        pdt[:, :Ws],
                        wd_t[:, ft * D_MODEL + dt * 128:
                             ft * D_MODEL + (dt + 1) * 128],
                        t_tiles[ft][:],
                        start=(ft == 0), stop=(ft == _FPG - 1))
                if g == 0:
                    nc.vector.tensor_copy(y_acc[s][dt][:], pdt[:, :Ws])
                else:
                    nc.vector.tensor_add(y_acc[s][dt][:], y_acc[s][dt][:],
                                         pdt[:, :Ws])
                    nc.scalar.dma_start(
                        dram[f"yt{s}"][:, dt * Ws:(dt + 1) * Ws],
                        y_acc[s][dt][:])

        prev = None
        for s, Ws in (("a", WA), ("b", WB)):
            for g in range(_NG):
                # group DMAs: chunks alternate gate/up in consumption
                # order; wd[g] (consumed one group later) last. Slot B's
                # x panel rides after slot A group 0's weights.
                wg_t, wu_t = [], []
                fine_g = fine_u = None
                for c in range(_NCH):
                    if s == "a" and g == 0 and c == 0:
                        # the very first weights stream as single f-tile
                        # pieces so the first real matmul starts ~1us
                        # sooner; chunk tile c0 goes unused this group.
                        fine_g, fine_u = [], []
                        for k in range(2):
                            fg = w0p.tile([128, D_MODEL], f16, tag=f"fg{k}",
                                          name=f"fine_g{k}")
                            nc.sync.dma_start(
                                fg[:], dram["wga"][0, 0, :,
                                                   k * D_MODEL:
                                                   (k + 1) * D_MODEL])
                            fu = w0p.tile([128, D_MODEL], f16, tag=f"fu{k}",
                                          name=f"fine_u{k}")
                            nc.sync.dma_start(
                                fu[:], dram["wua"][0, 0, :,
                                                   k * D_MODEL:
                                                   (k + 1) * D_MODEL])
                            fine_g.append(fg)
                            fine_u.append(fu)
                        wg_t.append(None)
                        wu_t.append(None)
                        continue
                    g_t = wgp.tile([128, _CHW], f16, tag=f"g{c}")
                    nc.sync.dma_start(g_t[:], dram[f"wg{s}"][g, c])
                    wg_t.append(g_t)
                    u_t = wup.tile([128, _CHW], f16, tag=f"u{c}")
                    nc.sync.dma_start(u_t[:], dram[f"wu{s}"][g, c])
                    wu_t.append(u_t)
                wd_t = wdp.tile([128, _FPG * D_MODEL], f16, tag="wd")
                nc.sync.dma_start(wd_t[:], dram[f"wd{s}"][g])
                if s == "a" and g == 0:
                    x_t["b"] = xpool.tile([128, _ND * WB], f16, tag="xb",
                                          name="x_b")
                    nc.sync.dma_start(x_t["b"][:], dram["xtb"][:])
                    emit_warmup(_WARM0)

                def g_stat(ft, d, _fg=fine_g):
                    if _fg is not None and ft < 2:
                        return _fg[ft][:, d * 128:(d + 1) * 128]
                    return wg_t[ft // 2][:, (ft % 2) * D_MODEL + d * 128:
                                         (ft % 2) * D_MODEL + (d + 1) * 128]

                def u_stat(ft, d, _fu=fine_u):
                    if _fu is not None and ft < 2:
                        return _fu[ft][:, d * 128:(d + 1) * 128]
                    return wu_t[ft // 2][:, (ft % 2) * D_MODEL + d * 128:
                                         (ft % 2) * D_MODEL + (d + 1) * 128]

                def x_mov(d):
                    return x_t[s][:, d * Ws:(d + 1) * Ws]

                t_tiles = []
                for ft in range(_FPG):
                    if s == "a" and g == 0 and ft < 2:
                        emit_warmup(_WARMI)
                    psg = pg.tile([128, WA], f32)
                    for d in range(_ND):
                        nc.tensor.matmul(psg[:, :Ws], g_stat(ft, d),
                                         x_mov(d),
                                         start=(d == 0), stop=(d == _ND - 1))
                    psu = pu.tile([128, WA], f32)
                    for d in range(_ND):
                        nc.tensor.matmul(psu[:, :Ws], u_stat(ft, d),
                                         x_mov(d),
                                         start=(d == 0), stop=(d == _ND - 1))
                    g_act = gap.tile([128, WA], f32, tag="gact")
                    nc.scalar.activation(g_act[:, :Ws], psg[:, :Ws],
                                         mybir.ActivationFunctionType.Silu)
                    t_t = tp.tile([128, Ws], f16, tag=f"t{s}{ft}")
                    nc.vector.tensor_mul(t_t[:], g_act[:, :Ws], psu[:, :Ws])
                    t_tiles.append(t_t)
                    if prev is not None:
                        emit_down(prev, (ft,))
                prev = (s, Ws, g, t_tiles, wd_t)

        # last group's down-projection (slot B group 1)
        for dt in range(_ND):
            emit_down(prev, (dt,))

    nc.compile()
    return nc


def _pack_gu(w):
    # [D, F] -> [F//1024, NCH, 128, CHW]; f-major within each group so
    # chunks arrive in exactly the order the f-tile loop consumes them
    a = np.asarray(w).astype(_np_cdt())
    D, F = a.shape
    ng = F // (_FPG * 128)
    a = a.reshape(_ND, 128, ng, _FPG, 128)       # [d, p, g, ft, f']
    a = a.transpose(2, 1, 3, 0, 4)               # [g, p, ft, d, f']
    a = a.reshape(ng, 128, _FPG * D_MODEL)       # col = ft*1024 + d*128 + f'
    a = a.reshape(ng, 128, _NCH, _CHW).transpose(0, 2, 1, 3)
    return np.ascontiguousarray(a)               # [g, c, p, CHW]


def _pack_wd(w):
    # [F, D] -> [F//1024, 128, FPG*D]; col = ft*1024 + j
    a = np.asarray(w).astype(_np_cdt())
    F, D = a.shape
    ng = F // (_FPG * 128)
    a = a.reshape(ng, _FPG, 128, D_MODEL).transpose(0, 2, 1, 3)
    return np.ascontiguousarray(a.reshape(ng, 128, _FPG * D_MODEL))


def _pack_x_panel(x_flat, toks, Ws):
    cdt = _np_cdt()
    xp = np.zeros((_ND, 128, Ws), dtype=cdt)
    if len(toks):
        xp[:, :, :len(toks)] = (
            x_flat[toks].T.astype(cdt).reshape(_ND, 128, len(toks)))
    return np.ascontiguousarray(xp.transpose(1, 0, 2).reshape(128, _ND * Ws))


def _plan(tok_lists):
    """Pair heavy experts with light ones; 2 cores per pair."""
    counts = np.array([len(t) for t in tok_lists])
    order = np.argsort(-counts, kind="stable")
    pairs = [(int(order[i]), int(order[N_EXPERTS - 1 - i]))
             for i in range(N_EXPERTS // 2)]
    WA = max(192, int(counts[order[0]]))
    WB = max(192, int(counts[order[N_EXPERTS // 2]]))
    return pairs, WA, WB


def _make_in_maps(pairs, WA, WB, tok_lists, x_flat, packed_w):
    """Core 2p: (heavy_p, F-half 0) + (light_p, F-half 1);
    core 2p+1 the complementary halves. packed_w[e] = (wg2, wu2, wd2)
    with the group axis spanning both halves (4 groups of 1024 f-cols:
    halves = groups [0:2] and [2:4])."""
    in_maps = []
    for p, (ea, eb) in enumerate(pairs):
        xa = _pack_x_panel(x_flat, tok_lists[ea], WA)
        xb = _pack_x_panel(x_flat, tok_lists[eb], WB)
        for half in (0, 1):
            ga, ua, da = packed_w[ea]
            gb, ub, db = packed_w[eb]
            ha = slice(2 * half, 2 * half + 2)
            hb = slice(2 * (1 - half), 2 * (1 - half) + 2)
            in_maps.append({
                "xta": xa, "xtb": xb,
                "wga": ga[ha], "wua": ua[ha], "wda": da[ha],
                "wgb": gb[hb], "wub": ub[hb], "wdb": db[hb],
            })
    return in_maps


def _combine(res, pairs, WA, WB, tok_lists, out_flat):
    for p, (ea, eb) in enumerate(pairs):
        ya = (np.asarray(res[2 * p]["yta"], dtype=np.float32)
              + np.asarray(res[2 * p + 1]["yta"], dtype=np.float32))
        yb = (np.asarray(res[2 * p]["ytb"], dtype=np.float32)
              + np.asarray(res[2 * p + 1]["ytb"], dtype=np.float32))
        for e, y, Ws in ((ea, ya, WA), (eb, yb, WB)):
            toks = tok_lists[e]
            if not len(toks):
                continue
            yf = y.reshape(128, _ND, Ws).transpose(1, 0, 2)
            out_flat[toks] = yf.reshape(D_MODEL, Ws)[:, :len(toks)].T


def _run_one(pairs, WA, WB, tok_lists, x_flat, packed_w, out_flat):
    from concourse.bass_utils import run_bass_kernel_spmd

    key = (WA, WB)
    if key not in _nc_cache:
        _nc_cache[key] = _build_nc(WA, WB)
    nc = _nc_cache[key]

    in_maps = _make_in_maps(pairs, WA, WB, tok_lists, x_flat, packed_w)

    res = None
    for attempt in range(3):
        try:
            res = run_bass_kernel_spmd(nc, in_maps,
                                       core_ids=list(range(N_CORES)))
            break
        except Exception:
            if attempt == 2:
                raise
            import time
            time.sleep(3.0)
            try:
                import jax
                jax.clear_caches()
                jax.clear_backends()
            except Exception:
                pass
    _combine(res.results, pairs, WA, WB, tok_lists, out_flat)


def kernel(x, expert_idx, w_gate, w_up, w_down):
    x = np.asarray(x, dtype=np.float32)
    idx = np.asarray(expert_idx).astype(np.int64)
    B, S, D = x.shape
    T = B * S
    x_flat = np.ascontiguousarray(x.reshape(T, D))
    idx_flat = idx.reshape(T)

    packed_w = [
        (_pack_gu(w_gate[e]), _pack_gu(w_up[e]), _pack_wd(w_down[e]))
        for e in range(N_EXPERTS)
    ]

    tok_lists = [np.nonzero(idx_flat == e)[0] for e in range(N_EXPERTS)]
    cap = max(1, max(len(t) for t in tok_lists))
    out_flat = np.zeros((T, D), dtype=np.float32)

    if cap <= 512:
        pairs, WA, WB = _plan(tok_lists)
        _run_one(pairs, WA, WB, tok_lists, x_flat, packed_w, out_flat)
    else:
        # fallback for extreme routing imbalance: rounds of <=512/expert
        rounds = -(-cap // 512)
        pairs = [(i, N_EXPERTS - 1 - i) for i in range(N_EXPERTS // 2)]
        for r in range(rounds):
            round_lists = [t[r * 512:(r + 1) * 512] for t in tok_lists]
            _run_one(pairs, 512, 512, round_lists, x_flat, packed_w,
                     out_flat)

    return out_flat.reshape(B, S, D)
